# revision 11
# baseline (speedup 1.0000x reference)
"""Trainium2 Bass kernel for nn_MultiHeadAttention (B=2, S=4096, D=1024, H=16, Dh=64).

Sharding over 8 cores: core c handles batch b=c//4 and head-group hg=c%4
(4 heads = 256 channels). Host gathers by summing the 4 per-head-group partial
output projections per batch (row-parallel output projection).

v2: ACT(exp)-saturated schedule.  The exp stream (512 x [128,1024] ACTIVATE,
~1.15us each = ~590us) is the hard floor; everything else hides behind it:
  - preamble only does K projection + first 2 Q chunks, so the first exp fires
    ~25-40us in (was ~101us).
  - V projection, remaining Q chunks and the output projection are issued as
    "filler" work interleaved into the attention kt-loop; the Tile scheduler
    pops them into PE gaps, which also keeps PE duty high so the HAM clock
    gate stays at K=8/8 (2.4 GHz).
  - per-(qb,pair) tail: attn_ps PSUM is released immediately via a cheap DVE
    copy to SBUF; the slow reciprocal (6.5us) and the normalize multiply run
    off the critical path on the SBUF copy.  No zero-fill matmuls (AV uses
    start=True on the first kt per PSUM quadrant).

Per-core device pipeline (all matmuls bf16, fp32 PSUM accumulation):
  QK^T:  lhsT=KT[64d,128k] rhs=QT[64d,1024q] -> ST [128k, 1024q] psum,
         two heads run concurrently on disjoint PE row-groups.
  exp:   ACT activation Exp (scale=1/8) PSUM->SBUF bf16  (ET [k,q])
  AV:    lhsT=V[128k,64d] rhs=ET[128k,512q] -> attnT [128d2, q] psum,
         two heads concurrent on disjoint PE col-groups.
  rowsum: DVE bf16 halving tree over k-chunks + PE ones-matmul 128->1.
  out projection out[t,o] = sum_c attnT[c,t] WoR[c,o] + bo.
"""

import math
import os
import sys
import functools

import numpy as np
import ml_dtypes

sys.path.insert(0, "/opt/trn_rl_repo")

import concourse.bass as bass  # noqa: E402
import concourse.mybir as mybir  # noqa: E402
import concourse.tile as tile  # noqa: E402
from concourse import bass_utils  # noqa: E402

B, S, D, H, DH = 2, 4096, 1024, 16, 64
NCORES = 8
HG = 4  # head groups (cores per batch)
OC = 256  # q/k/v channels per core
BF16 = mybir.dt.bfloat16
F32 = mybir.dt.float32
QBLK = 1024
NQB = S // QBLK  # 4
NKT = S // 128  # 32 k-tiles
NTT = S // 128  # 32 t-tiles
bf16 = ml_dtypes.bfloat16


_TPB_ENGINES = None


def _split_waits(nc, max_waits=1):
    """walrus codegen in this container rejects TPB instructions carrying more
    than one sync-wait command.  Spill extra semaphore waits onto preceding
    NoOps on the same engine (engines execute their queue in order, so a NoOp
    that waits immediately before the instruction is equivalent)."""
    import bass_rust

    global _TPB_ENGINES
    if _TPB_ENGINES is None:
        _TPB_ENGINES = {
            mybir.EngineType.Pool,
            mybir.EngineType.Activation,
            mybir.EngineType.PE,
            mybir.EngineType.DVE,
            mybir.EngineType.SP,
        }
    ctr = 0
    for bb in nc.main_func.blocks:
        insts = bb.instructions
        out = []
        changed = False
        for inst in insts:
            si = getattr(inst, "sync_info", None)
            if (
                si is not None
                and si.on_wait
                and len(si.on_wait) > max_waits
                and inst.engine in _TPB_ENGINES
            ):
                waits = list(si.on_wait)
                keep = waits[-max_waits:]
                spill = waits[:-max_waits]
                for i in range(0, len(spill), max_waits):
                    nop = bass_rust.InstNoOp(
                        name=f"{inst.name}-sw{ctr}", ins=[], outs=[]
                    )
                    ctr += 1
                    nop.engine = inst.engine
                    nop.sync_info = mybir.SyncInfo(
                        on_wait=spill[i : i + max_waits], on_update=[]
                    )
                    out.append(nop)
                inst.sync_info = mybir.SyncInfo(
                    on_wait=keep, on_update=list(si.on_update)
                )
                changed = True
            out.append(inst)
        if changed:
            insts[:] = out
    return nc


@functools.lru_cache(maxsize=4)
def _build(masked: bool, split_waits: bool = True):
    nc = bass.Bass()

    xqT_d = nc.dram_tensor("xqT", [D, S], BF16, kind="ExternalInput")
    xkT_d = nc.dram_tensor("xkT", [D, S], BF16, kind="ExternalInput")
    xvT_d = nc.dram_tensor("xvT", [D, S], BF16, kind="ExternalInput")
    wqT_d = nc.dram_tensor("wqT", [D, OC], BF16, kind="ExternalInput")
    wkT_d = nc.dram_tensor("wkT", [D, OC], BF16, kind="ExternalInput")
    wvT_d = nc.dram_tensor("wvT", [D, OC], BF16, kind="ExternalInput")
    bq_d = nc.dram_tensor("bq2", [128, 2], F32, kind="ExternalInput")
    bk_d = nc.dram_tensor("bk2", [128, 2], F32, kind="ExternalInput")
    bv_d = nc.dram_tensor("bvr", [1, OC], F32, kind="ExternalInput")
    woR_d = nc.dram_tensor("woR", [OC, D], BF16, kind="ExternalInput")
    bo_d = nc.dram_tensor("bor", [1, D], F32, kind="ExternalInput")
    if masked:
        maskT_d = nc.dram_tensor("maskT", [S, S], BF16, kind="ExternalInput")
    out_d = nc.dram_tensor("out", [S, D], F32, kind="ExternalOutput")

    with tile.TileContext(nc) as tc:
        with (
            tc.tile_pool(name="persist", bufs=1) as persist,
            tc.tile_pool(name="wpool", bufs=1) as wpool,
            tc.tile_pool(name="xt", bufs=2) as xtp,
            tc.tile_pool(name="et", bufs=5) as etp,
            tc.tile_pool(name="rtmp", bufs=3) as rtmp,
            tc.tile_pool(name="rrow", bufs=2) as rrow,
            tc.tile_pool(name="atu", bufs=2) as atup,
            tc.tile_pool(name="outp", bufs=2) as outp,
            tc.tile_pool(name="stp", bufs=3, space="PSUM") as stp,
            tc.tile_pool(name="avps", bufs=1, space="PSUM") as avps,
        ):
            # persistent SBUF tensors
            QT = [persist.tile([128, S], BF16, tag=f"QT{p}", name=f"QT{p}") for p in range(2)]
            KT = [persist.tile([128, S], BF16, tag=f"KT{p}", name=f"KT{p}") for p in range(2)]
            Vsb = persist.tile([128, NKT, OC], BF16, tag="Vsb")
            attnT = [persist.tile([128, S], BF16, tag=f"attnT{p}", name=f"attnT{p}") for p in range(2)]
            ones_row = persist.tile([1, 128], F32, tag="ones_row")
            ones64 = persist.tile([128, 64], BF16, tag="ones64")
            bv_bc = persist.tile([128, OC], F32, tag="bv_bc")
            bo_bc = persist.tile([128, D], F32, tag="bo_bc")
            nc.gpsimd.memset(ones_row[:], 1.0)
            nc.gpsimd.memset(ones64[:], 1.0)

            wq_sb = wpool.tile([128, 8, OC], BF16, tag="wq")
            wk_sb = wpool.tile([128, 8, OC], BF16, tag="wk")
            wv_sb = wpool.tile([128, 8, OC], BF16, tag="wv")
            wo_sb = wpool.tile([128, 2, D], BF16, tag="wo")
            bq_sb = wpool.tile([128, 2], F32, tag="bq")
            bk_sb = wpool.tile([128, 2], F32, tag="bk")
            bv_sb = wpool.tile([1, OC], F32, tag="bv")
            bo_sb = wpool.tile([1, D], F32, tag="bo")
            nc.sync.dma_start(wq_sb[:], wqT_d.rearrange("(dc p) o -> p dc o", p=128))
            nc.sync.dma_start(wk_sb[:], wkT_d.rearrange("(dc p) o -> p dc o", p=128))
            nc.sync.dma_start(wv_sb[:], wvT_d.rearrange("(dc p) o -> p dc o", p=128))
            nc.sync.dma_start(wo_sb[:], woR_d.rearrange("(cc p) o -> p cc o", p=128))
            nc.sync.dma_start(bq_sb[:], bq_d[:])
            nc.sync.dma_start(bk_sb[:], bk_d[:])
            nc.sync.dma_start(bv_sb[:], bv_d[:])
            nc.sync.dma_start(bo_sb[:], bo_d[:])

            # priority offset for the attention-critical stream: QK^T, exp,
            # AV and the attn_ps release copy are emitted ahead of any
            # DMA-gated projection/out-proj filler work, which the scheduler
            # then packs into real gaps only.
            HIP = 1 << 20

            # broadcast bv / bo across partitions via K=1 matmuls
            with tc.high_priority(offset=HIP):
                ps = stp.tile([128, QBLK], F32, tag="st", name="bvbc")
                nc.tensor.matmul(ps[:, 0:OC], ones_row[:], bv_sb[:], start=True, stop=True)
                nc.vector.tensor_copy(bv_bc[:], ps[:, 0:OC])
                ps = stp.tile([128, QBLK], F32, tag="st", name="bobc")
                for oh in range(2):
                    nc.tensor.matmul(
                        ps[:, oh * 512 : (oh + 1) * 512],
                        ones_row[:],
                        bo_sb[:, oh * 512 : (oh + 1) * 512],
                        start=True,
                        stop=True,
                    )
                nc.vector.tensor_copy(bo_bc[:], ps[:])

            xqr = xqT_d.rearrange("(dc p) t -> p dc t", p=128)
            xkr = xkT_d.rearrange("(dc p) t -> p dc t", p=128)
            xvr = xvT_d.rearrange("(dc p) t -> p dc t", p=128)

            def qk_chunk(xr, wsb, bsb, dst, oc, tt, xt=None):
                """one 512-token projection chunk for q or k, one oc half"""
                if xt is None:
                    xt = xtp.tile([128, 8, 512], BF16, tag="xqk", name="xqk")
                    nc.sync.dma_start(xt[:], xr[:, :, tt * 512 : (tt + 1) * 512])
                pst = stp.tile([128, QBLK], F32, tag="st", name="pqk")
                for dc in range(8):
                    nc.tensor.matmul(
                        pst[:, 0:512],
                        wsb[:, dc, oc * 128 : (oc + 1) * 128],
                        xt[:, dc, :],
                        start=(dc == 0),
                        stop=(dc == 7),
                    )
                nc.vector.tensor_scalar_add(
                    dst[oc][:, tt * 512 : (tt + 1) * 512],
                    pst[:, 0:512],
                    bsb[:, oc : oc + 1],
                )
                return xt

            def v_chunk(tcI):
                xt = xtp.tile([128, 8, 128], BF16, tag="xv", name="xv")
                nc.sync.dma_start(xt[:], xvr[:, :, tcI * 128 : (tcI + 1) * 128])
                pst = stp.tile([128, QBLK], F32, tag="st", name="pv")
                for dc in range(8):
                    nc.tensor.matmul(
                        pst[:, 0:OC],
                        xt[:, dc, :],
                        wv_sb[:, dc, :],
                        start=(dc == 0),
                        stop=(dc == 7),
                    )
                nc.vector.tensor_add(Vsb[:, tcI, :], pst[:, 0:OC], bv_bc[:])

            our = out_d.rearrange("(tt p) o -> tt p o", p=128)

            def out_chunk(tt):
                ps = stp.tile([128, QBLK], F32, tag="st", name="po")
                for oh in range(2):
                    for cc in range(2):
                        nc.tensor.matmul(
                            ps[:, oh * 512 : (oh + 1) * 512],
                            attnT[cc][:, tt * 128 : (tt + 1) * 128],
                            wo_sb[:, cc, oh * 512 : (oh + 1) * 512],
                            start=(cc == 0),
                            stop=(cc == 1),
                        )
                ot = outp.tile([128, D], F32, tag="ot", name="ot")
                nc.vector.tensor_add(ot[:], ps[:], bo_bc[:])
                nc.sync.dma_start(our[tt], ot[:])

            # ---------------- preamble projections ----------------
            # minimum work before the first QK^T/exp can fire: Q chunks for
            # qb0/pair0 and the first K chunk (k-tiles 0..3) for both pairs.
            with tc.high_priority(offset=HIP):
                for tt in range(2):
                    qk_chunk(xqr, wq_sb, bq_sb, QT, 0, tt)
                xt = qk_chunk(xkr, wk_sb, bk_sb, KT, 0, 0)
                qk_chunk(xkr, wk_sb, bk_sb, KT, 1, 0, xt=xt)
            # V prologue: first 16 k-chunks (filler priority)
            for tcI in range(16):
                v_chunk(tcI)

            # ---------------- filler plan ----------------
            # per-segment (seg = qb*2 + pair) explicit (kt -> item) schedule;
            # items are popped at their kt position inside the attention loop
            # and filled into PE/DVE gaps by the scheduler.  Placement keeps
            # each producer >=2 kt ahead of its first consumer.
            seg_fillers = {s: [] for s in range(2 * NQB)}
            seg_fillers[0] = (
                [(0, ("k", 0, 1)), (1, ("k", 0, 2)), (5, ("k", 0, 3)),
                 (9, ("k", 0, 4)), (13, ("k", 0, 5)), (17, ("k", 0, 6)),
                 (21, ("k", 0, 7))]
                + [(2, ("v", 16)), (3, ("v", 17)), (4, ("v", 18)), (6, ("v", 19)),
                   (7, ("v", 20)), (8, ("v", 21)), (10, ("v", 22)), (11, ("v", 23)),
                   (12, ("v", 24)), (14, ("v", 25)), (15, ("v", 26)), (16, ("v", 27)),
                   (18, ("v", 28)), (19, ("v", 29)), (20, ("v", 30)), (22, ("v", 31))]
                + [(24, ("q", 1, 0)), (26, ("q", 1, 1))]
            )
            seg_fillers[1] = [(0, ("k", 1, 1)), (2, ("k", 1, 2)), (4, ("k", 1, 3)),
                              (8, ("k", 1, 4)), (12, ("k", 1, 5)),
                              (16, ("k", 1, 6)), (20, ("k", 1, 7)),
                              (22, ("q", 0, 2)), (24, ("q", 0, 3))]
            seg_fillers[2] = [(4 * tt, ("o", 0, tt)) for tt in range(8)] + [
                (2, ("q", 1, 2)), (6, ("q", 1, 3))]
            seg_fillers[3] = [(8, ("q", 0, 4)), (12, ("q", 0, 5))]
            seg_fillers[4] = [(4 * tt, ("o", 1, tt)) for tt in range(8)] + [
                (2, ("q", 1, 4)), (6, ("q", 1, 5))]
            seg_fillers[5] = [(8, ("q", 0, 6)), (12, ("q", 0, 7))]
            seg_fillers[6] = [(4 * tt, ("o", 2, tt)) for tt in range(8)] + [
                (2, ("q", 1, 6)), (6, ("q", 1, 7))]
            seg_fillers[7] = []

            def run_filler(item):
                kind = item[0]
                if kind == "q":
                    qk_chunk(xqr, wq_sb, bq_sb, QT, item[1], item[2])
                elif kind == "k":
                    qk_chunk(xkr, wk_sb, bk_sb, KT, item[1], item[2])
                elif kind == "v":
                    v_chunk(item[1])
                elif kind == "o":
                    out_chunk(item[1] * 8 + item[2])

            # ---------------- attention ----------------
            if masked:
                mrr = maskT_d.rearrange("(kt p) q -> kt p q", p=128)
            for qb in range(NQB):
                q0 = qb * QBLK
                for pair in range(2):
                    seg = qb * 2 + pair
                    fpos = {}
                    for kt_pos, item in seg_fillers[seg]:
                        fpos.setdefault(kt_pos, []).append(item)
                    QTp, KTp, ATp = QT[pair], KT[pair], attnT[pair]
                    attn_ps = avps.tile([128, QBLK], F32, tag="attn", name="attn")
                    rs5 = [None, None]
                    et_cur = [None, None]
                    for kt in range(NKT):
                        for item in fpos.get(kt, ()):
                            run_filler(item)
                        ki = kt % 4
                        with tc.high_priority(offset=HIP):
                            sts = [None, None]
                            for h2 in range(2):
                                if ki == 0:
                                    et_cur[h2] = etp.tile(
                                        [128, 4, QBLK], BF16, tag="et", name="et"
                                    )
                                sts[h2] = stp.tile([128, QBLK], F32, tag="st", name="st")
                            for qh in range(2):
                                for h2 in range(2):
                                    b0 = h2 * 64
                                    nc.tensor.matmul(
                                        sts[h2][:, qh * 512 : (qh + 1) * 512],
                                        KTp[b0 : b0 + 64, kt * 128 : (kt + 1) * 128],
                                        QTp[b0 : b0 + 64, q0 + qh * 512 : q0 + (qh + 1) * 512],
                                        start=True,
                                        stop=True,
                                    )
                            for h2 in range(2):
                                et = et_cur[h2]
                                nc.scalar.activation(
                                    et[:, ki, :],
                                    sts[h2][:],
                                    mybir.ActivationFunctionType.Exp,
                                    scale=1.0 / math.sqrt(DH),
                                )
                                if masked:
                                    mk = rtmp.tile([128, QBLK], BF16, tag="mk", name="mk")
                                    nc.sync.dma_start(mk[:], mrr[kt][:, q0 : q0 + QBLK])
                                    nc.vector.tensor_mul(et[:, ki, :], et[:, ki, :], mk[:])
                            # AV accumulate (start on first kt per quadrant)
                            for qh in range(2):
                                for h2 in range(2):
                                    b0 = h2 * 64
                                    h_local = pair * 2 + h2
                                    nc.tensor.matmul(
                                        attn_ps[b0 : b0 + 64, qh * 512 : (qh + 1) * 512],
                                        Vsb[:, kt, h_local * 64 : (h_local + 1) * 64],
                                        et_cur[h2][:, ki, qh * 512 : (qh + 1) * 512],
                                        start=(kt == 0),
                                        stop=(kt == NKT - 1),
                                    )
                        # rowsum tree contribution once per 4-k-tile group
                        # (last group deferred past the attn_ps release copy)
                        if ki == 3 and kt != NKT - 1:
                            for h2 in range(2):
                                et = et_cur[h2]
                                t2 = rtmp.tile([128, 2, QBLK], BF16, tag="t2", name="t2")
                                nc.vector.tensor_add(t2[:], et[:, 0:2, :], et[:, 2:4, :])
                                if kt == 3:
                                    rs5[h2] = rtmp.tile(
                                        [128, QBLK], BF16, tag="rs5", name="rs5"
                                    )
                                    nc.vector.tensor_add(rs5[h2][:], t2[:, 0, :], t2[:, 1, :])
                                else:
                                    ts_ = rtmp.tile([128, QBLK], BF16, tag="ts", name="ts")
                                    nc.vector.tensor_add(ts_[:], t2[:, 0, :], t2[:, 1, :])
                                    nc.vector.tensor_add(rs5[h2][:], rs5[h2][:], ts_[:])
                    # ---- pair tail ----
                    # release attn_ps ASAP: cheap DVE copy to SBUF, at top
                    # priority so pending tree adds don't delay it (next
                    # pair's first AV waits on this).
                    atu = atup.tile([128, QBLK], F32, tag="atu", name="atu")
                    with tc.high_priority(offset=2 * HIP):
                        nc.vector.tensor_copy(atu[:], attn_ps[:])
                    # deferred rowsum tree for the last 4-k-tile group
                    for h2 in range(2):
                        et = et_cur[h2]
                        t2 = rtmp.tile([128, 2, QBLK], BF16, tag="t2", name="t2")
                        nc.vector.tensor_add(t2[:], et[:, 0:2, :], et[:, 2:4, :])
                        ts_ = rtmp.tile([128, QBLK], BF16, tag="ts", name="ts")
                        nc.vector.tensor_add(ts_[:], t2[:, 0, :], t2[:, 1, :])
                        nc.vector.tensor_add(rs5[h2][:], rs5[h2][:], ts_[:])
                    # rowsum 128->1 + broadcast via ones[128,64] matmuls
                    st_rs = stp.tile([128, QBLK], F32, tag="st", name="st_rs")
                    for qh in range(2):
                        for h2 in range(2):
                            qx = (qh + h2) % 2
                            nc.tensor.matmul(
                                st_rs[h2 * 64 : h2 * 64 + 64, qx * 512 : qx * 512 + 512],
                                ones64[:],
                                rs5[h2][:, qx * 512 : qx * 512 + 512],
                                start=True,
                                stop=True,
                            )
                    rs_bc = rrow.tile([128, QBLK], F32, tag="rsbc", name="rsbc")
                    nc.vector.tensor_copy(rs_bc[:], st_rs[:])
                    # reciprocal + normalize in halves so the tail pipelines
                    # (out-proj of a half can start while the other half runs)
                    for hf in range(2):
                        sl = slice(hf * 512, (hf + 1) * 512)
                        nc.vector.reciprocal(rs_bc[:, sl], rs_bc[:, sl])
                        nc.vector.tensor_tensor(
                            ATp[:, q0 + hf * 512 : q0 + (hf + 1) * 512],
                            atu[:, sl],
                            rs_bc[:, sl],
                            mybir.AluOpType.mult,
                        )
            # tail: output projection for the last q-block
            for tt in range(3 * 8, 4 * 8):
                out_chunk(tt)

    return _split_waits(nc) if split_waits else nc


def _prep_in_maps(inputs):
    q = np.asarray(inputs["query"], np.float32)
    k = np.asarray(inputs["key"], np.float32)
    v = np.asarray(inputs["value"], np.float32)
    mask = np.asarray(inputs["mask"])
    Wq = np.asarray(inputs["Wq"], np.float32)
    Wk = np.asarray(inputs["Wk"], np.float32)
    Wv = np.asarray(inputs["Wv"], np.float32)
    Wo = np.asarray(inputs["Wo"], np.float32)
    bq = np.asarray(inputs["bq"], np.float32)
    bk = np.asarray(inputs["bk"], np.float32)
    bv = np.asarray(inputs["bv"], np.float32)
    bo = np.asarray(inputs["bo"], np.float32)

    masked = not bool((mask != 0).all())
    xT = {}
    for nm, x in (("q", q), ("k", k), ("v", v)):
        for b in range(B):
            xT[(nm, b)] = np.ascontiguousarray(x[b].T).astype(bf16)
    if masked:
        maskT = np.ascontiguousarray(
            (np.broadcast_to(mask[0, 0], (S, S)).T != 0)
        ).astype(bf16)

    in_maps = []
    for c in range(NCORES):
        b, hg = c // HG, c % HG
        sl = slice(hg * OC, (hg + 1) * OC)
        m = {
            "xqT": xT[("q", b)],
            "xkT": xT[("k", b)],
            "xvT": xT[("v", b)],
            "wqT": np.ascontiguousarray(Wq[sl].T).astype(bf16),
            "wkT": np.ascontiguousarray(Wk[sl].T).astype(bf16),
            "wvT": np.ascontiguousarray(Wv[sl].T).astype(bf16),
            "bq2": np.ascontiguousarray(bq[sl].reshape(2, 128).T),
            "bk2": np.ascontiguousarray(bk[sl].reshape(2, 128).T),
            "bvr": bv[sl].reshape(1, OC).copy(),
            "woR": np.ascontiguousarray(Wo[:, sl].T).astype(bf16),
            "bor": (bo if hg == 0 else np.zeros_like(bo)).reshape(1, D).copy(),
        }
        if masked:
            m["maskT"] = maskT
        in_maps.append(m)
    return in_maps, masked


def _install_profile_hook():
    """Provide antenv.axon_hooks + register the NTFF profile hook via ctypes
    against libaxon_pjrt.so (the agent image lacks antenv.axon_hooks, which
    makes run_bass_kernel_spmd(trace=True) fall over; see trn_boot.py)."""
    import types
    import ctypes
    import contextlib

    if "antenv.axon_hooks" in sys.modules:
        return
    mod = types.ModuleType("antenv.axon_hooks")
    state = {"hook": None}
    mod.set_axon_ntff_profile_hook = lambda h: state.__setitem__("hook", h)
    mod.get_axon_ntff_profile_hook = lambda: state["hook"]
    sys.modules["antenv.axon_hooks"] = mod

    so_path = "/opt/axon/libaxon_pjrt.so"
    if not os.path.exists(so_path):
        return
    lib = ctypes.CDLL(so_path)
    if not hasattr(lib, "axon_start_nrt_profile"):
        return
    lib.axon_start_nrt_profile.argtypes = [
        ctypes.POINTER(ctypes.c_int64),
        ctypes.c_size_t,
    ]
    lib.axon_start_nrt_profile.restype = ctypes.c_int64
    lib.axon_stop_nrt_profile.argtypes = [ctypes.c_char_p]
    lib.axon_stop_nrt_profile.restype = ctypes.c_int64

    @contextlib.contextmanager
    def _hook(output_dir, device_ids):
        import jax

        jax.devices()
        if device_ids:
            ids = (ctypes.c_int64 * len(device_ids))(*device_ids)
            rc = lib.axon_start_nrt_profile(ids, len(device_ids))
        else:
            rc = lib.axon_start_nrt_profile(None, 0)
        if rc != 0:
            raise RuntimeError(f"axon_start_nrt_profile rc={rc}")
        try:
            yield
        finally:
            n = lib.axon_stop_nrt_profile(str(output_dir).encode())
            print(f"profile: {n} file(s) written to {output_dir}", file=sys.stderr)

    mod.set_axon_ntff_profile_hook(_hook)


def run(inputs, trace=False):
    if trace:
        _install_profile_hook()
    in_maps, masked = _prep_in_maps(inputs)
    nc = _build(masked)
    res = bass_utils.run_bass_kernel_spmd(
        nc, in_maps, core_ids=list(range(NCORES)), trace=trace
    )
    out = np.zeros((B, S, D), np.float32)
    for c in range(NCORES):
        out[c // HG] += res.results[c]["out"]
    return out, res


def kernel(**inputs):
    return run(inputs, trace=False)[0]


# revision 16
# speedup vs baseline: 1.2122x; 1.2122x over previous
"""Trainium2 Bass kernel for nn_MultiHeadAttention (B=2, S=4096, D=1024, H=16, Dh=64).

Sharding over 8 cores: core c handles batch b=c//4 and head-group hg=c%4
(4 heads = 256 channels). Host gathers by summing the 4 per-head-group partial
output projections per batch (row-parallel output projection).

v2: ACT(exp)-saturated schedule.  The exp stream (512 x [128,1024] ACTIVATE,
~1.15us each = ~590us) is the hard floor; everything else hides behind it:
  - preamble only does K projection + first 2 Q chunks, so the first exp fires
    ~25-40us in (was ~101us).
  - V projection, remaining Q chunks and the output projection are issued as
    "filler" work interleaved into the attention kt-loop; the Tile scheduler
    pops them into PE gaps, which also keeps PE duty high so the HAM clock
    gate stays at K=8/8 (2.4 GHz).
  - per-(qb,pair) tail: attn_ps PSUM is released immediately via a cheap DVE
    copy to SBUF; the slow reciprocal (6.5us) and the normalize multiply run
    off the critical path on the SBUF copy.  No zero-fill matmuls (AV uses
    start=True on the first kt per PSUM quadrant).

Per-core device pipeline (all matmuls bf16, fp32 PSUM accumulation):
  QK^T:  lhsT=KT[64d,128k] rhs=QT[64d,1024q] -> ST [128k, 1024q] psum,
         two heads run concurrently on disjoint PE row-groups.
  exp:   ACT activation Exp (scale=1/8) PSUM->SBUF bf16  (ET [k,q])
  AV:    lhsT=V[128k,64d] rhs=ET[128k,512q] -> attnT [128d2, q] psum,
         two heads concurrent on disjoint PE col-groups.
  rowsum: DVE bf16 halving tree over k-chunks + PE ones-matmul 128->1.
  out projection out[t,o] = sum_c attnT[c,t] WoR[c,o] + bo.
"""

import math
import os
import sys
import functools

import numpy as np
import ml_dtypes

sys.path.insert(0, "/opt/trn_rl_repo")

import concourse.bass as bass  # noqa: E402
import concourse.mybir as mybir  # noqa: E402
import concourse.tile as tile  # noqa: E402
from concourse import bass_utils  # noqa: E402

B, S, D, H, DH = 2, 4096, 1024, 16, 64
NCORES = 8
HG = 4  # head groups (cores per batch)
OC = 256  # q/k/v channels per core
BF16 = mybir.dt.bfloat16
F32 = mybir.dt.float32
QBLK = 1024
NQB = S // QBLK  # 4
NKT = S // 128  # 32 k-tiles
NTT = S // 128  # 32 t-tiles
bf16 = ml_dtypes.bfloat16


_TPB_ENGINES = None


def _split_waits(nc, max_waits=1):
    """walrus codegen in this container rejects TPB instructions carrying more
    than one sync-wait command.  Spill extra semaphore waits onto preceding
    NoOps on the same engine (engines execute their queue in order, so a NoOp
    that waits immediately before the instruction is equivalent)."""
    import bass_rust

    global _TPB_ENGINES
    if _TPB_ENGINES is None:
        _TPB_ENGINES = {
            mybir.EngineType.Pool,
            mybir.EngineType.Activation,
            mybir.EngineType.PE,
            mybir.EngineType.DVE,
            mybir.EngineType.SP,
        }
    ctr = 0
    for bb in nc.main_func.blocks:
        insts = bb.instructions
        out = []
        changed = False
        for inst in insts:
            si = getattr(inst, "sync_info", None)
            if (
                si is not None
                and si.on_wait
                and len(si.on_wait) > max_waits
                and inst.engine in _TPB_ENGINES
            ):
                waits = list(si.on_wait)
                keep = waits[-max_waits:]
                spill = waits[:-max_waits]
                for i in range(0, len(spill), max_waits):
                    nop = bass_rust.InstNoOp(
                        name=f"{inst.name}-sw{ctr}", ins=[], outs=[]
                    )
                    ctr += 1
                    nop.engine = inst.engine
                    nop.sync_info = mybir.SyncInfo(
                        on_wait=spill[i : i + max_waits], on_update=[]
                    )
                    out.append(nop)
                inst.sync_info = mybir.SyncInfo(
                    on_wait=keep, on_update=list(si.on_update)
                )
                changed = True
            out.append(inst)
        if changed:
            insts[:] = out
    return nc


@functools.lru_cache(maxsize=4)
def _build(masked: bool, split_waits: bool = True):
    nc = bass.Bass()

    xqT_d = nc.dram_tensor("xqT", [D, S], BF16, kind="ExternalInput")
    xkT_d = nc.dram_tensor("xkT", [D, S], BF16, kind="ExternalInput")
    xvT_d = nc.dram_tensor("xvT", [D, S], BF16, kind="ExternalInput")
    wqT_d = nc.dram_tensor("wqT", [D, OC], BF16, kind="ExternalInput")
    wkT_d = nc.dram_tensor("wkT", [D, OC], BF16, kind="ExternalInput")
    wvT_d = nc.dram_tensor("wvT", [D, OC], BF16, kind="ExternalInput")
    bq_d = nc.dram_tensor("bq2", [128, 2], F32, kind="ExternalInput")
    bk_d = nc.dram_tensor("bk2", [128, 2], F32, kind="ExternalInput")
    bv_d = nc.dram_tensor("bvr", [1, OC], F32, kind="ExternalInput")
    woR_d = nc.dram_tensor("woR", [OC, D], BF16, kind="ExternalInput")
    bo_d = nc.dram_tensor("bor", [1, D], F32, kind="ExternalInput")
    if masked:
        maskT_d = nc.dram_tensor("maskT", [S, S], BF16, kind="ExternalInput")
    out_d = nc.dram_tensor("out", [S, D], F32, kind="ExternalOutput")

    with tile.TileContext(nc) as tc:
        with (
            tc.tile_pool(name="persist", bufs=1) as persist,
            tc.tile_pool(name="wpool", bufs=1) as wpool,
            tc.tile_pool(name="xt", bufs=2) as xtp,
            tc.tile_pool(name="et", bufs=5) as etp,
            tc.tile_pool(name="rtmp", bufs=3) as rtmp,
            tc.tile_pool(name="rrow", bufs=2) as rrow,
            tc.tile_pool(name="atu", bufs=2) as atup,
            tc.tile_pool(name="outp", bufs=2) as outp,
            tc.tile_pool(name="stp", bufs=3, space="PSUM") as stp,
            tc.tile_pool(name="avps", bufs=1, space="PSUM") as avps,
        ):
            # persistent SBUF tensors
            QT = [persist.tile([128, S], BF16, tag=f"QT{p}", name=f"QT{p}") for p in range(2)]
            KT = [persist.tile([128, S], BF16, tag=f"KT{p}", name=f"KT{p}") for p in range(2)]
            Vsb = persist.tile([128, NKT, OC], BF16, tag="Vsb")
            attnT = [persist.tile([128, S], BF16, tag=f"attnT{p}", name=f"attnT{p}") for p in range(2)]
            ones_row = persist.tile([1, 128], F32, tag="ones_row")
            ones64 = persist.tile([128, 64], BF16, tag="ones64")
            bv_bc = persist.tile([128, OC], F32, tag="bv_bc")
            bo_bc = persist.tile([128, D], F32, tag="bo_bc")
            nc.gpsimd.memset(ones_row[:], 1.0)
            nc.gpsimd.memset(ones64[:], 1.0)

            wq_sb = wpool.tile([128, 8, OC], BF16, tag="wq")
            wk_sb = wpool.tile([128, 8, OC], BF16, tag="wk")
            wv_sb = wpool.tile([128, 8, OC], BF16, tag="wv")
            wo_sb = wpool.tile([128, 2, D], BF16, tag="wo")
            bq_sb = wpool.tile([128, 2], F32, tag="bq")
            bk_sb = wpool.tile([128, 2], F32, tag="bk")
            bv_sb = wpool.tile([1, OC], F32, tag="bv")
            bo_sb = wpool.tile([1, D], F32, tag="bo")
            nc.sync.dma_start(wq_sb[:], wqT_d.rearrange("(dc p) o -> p dc o", p=128))
            nc.sync.dma_start(wk_sb[:], wkT_d.rearrange("(dc p) o -> p dc o", p=128))
            nc.sync.dma_start(wv_sb[:], wvT_d.rearrange("(dc p) o -> p dc o", p=128))
            nc.sync.dma_start(wo_sb[:], woR_d.rearrange("(cc p) o -> p cc o", p=128))
            nc.sync.dma_start(bq_sb[:], bq_d[:])
            nc.sync.dma_start(bk_sb[:], bk_d[:])
            nc.sync.dma_start(bv_sb[:], bv_d[:])
            nc.sync.dma_start(bo_sb[:], bo_d[:])

            # PE warmup: ~7us of dense dummy matmuls right at kernel start so
            # the HAM clock gate reaches K=8/8 (2.4 GHz) before the real
            # (DMA-gated) projection matmuls run.  Output is never read.
            warm_sb = persist.tile([128, 512], BF16, tag="warm")
            nc.gpsimd.memset(warm_sb[:], 0.0)
            wps = stp.tile([128, QBLK], F32, tag="st", name="warmps")
            for _ in range(28):
                nc.tensor.matmul(
                    wps[:, 0:512], warm_sb[:, 0:128], warm_sb[:], start=True, stop=True
                )
            warm_out = persist.tile([1, 8], F32, tag="warm_out")
            nc.vector.tensor_copy(warm_out[:], wps[0:1, 0:8])

            # broadcast bv / bo across partitions via K=1 matmuls
            ps = stp.tile([128, QBLK], F32, tag="st", name="bvbc")
            nc.tensor.matmul(ps[:, 0:OC], ones_row[:], bv_sb[:], start=True, stop=True)
            nc.vector.tensor_copy(bv_bc[:], ps[:, 0:OC])
            ps = stp.tile([128, QBLK], F32, tag="st", name="bobc")
            for oh in range(2):
                nc.tensor.matmul(
                    ps[:, oh * 512 : (oh + 1) * 512],
                    ones_row[:],
                    bo_sb[:, oh * 512 : (oh + 1) * 512],
                    start=True,
                    stop=True,
                )
            nc.vector.tensor_copy(bo_bc[:], ps[:])

            xqr = xqT_d.rearrange("(dc p) t -> p dc t", p=128)
            xkr = xkT_d.rearrange("(dc p) t -> p dc t", p=128)
            xvr = xvT_d.rearrange("(dc p) t -> p dc t", p=128)

            def qk_chunk(xr, wsb, bsb, dst, oc, tt, xt=None):
                """one 512-token projection chunk for q or k, one oc half"""
                if xt is None:
                    xt = xtp.tile([128, 8, 512], BF16, tag="xqk", name="xqk")
                    nc.sync.dma_start(xt[:], xr[:, :, tt * 512 : (tt + 1) * 512])
                pst = stp.tile([128, QBLK], F32, tag="st", name="pqk")
                for dc in range(8):
                    nc.tensor.matmul(
                        pst[:, 0:512],
                        wsb[:, dc, oc * 128 : (oc + 1) * 128],
                        xt[:, dc, :],
                        start=(dc == 0),
                        stop=(dc == 7),
                    )
                nc.vector.tensor_scalar_add(
                    dst[oc][:, tt * 512 : (tt + 1) * 512],
                    pst[:, 0:512],
                    bsb[:, oc : oc + 1],
                )
                return xt

            def v_chunk(tcI):
                xt = xtp.tile([128, 8, 128], BF16, tag="xv", name="xv")
                nc.sync.dma_start(xt[:], xvr[:, :, tcI * 128 : (tcI + 1) * 128])
                pst = stp.tile([128, QBLK], F32, tag="st", name="pv")
                for dc in range(8):
                    nc.tensor.matmul(
                        pst[:, 0:OC],
                        xt[:, dc, :],
                        wv_sb[:, dc, :],
                        start=(dc == 0),
                        stop=(dc == 7),
                    )
                nc.vector.tensor_add(Vsb[:, tcI, :], pst[:, 0:OC], bv_bc[:])

            our = out_d.rearrange("(tt p) o -> tt p o", p=128)

            def out_chunk(tt):
                ps = stp.tile([128, QBLK], F32, tag="st", name="po")
                for oh in range(2):
                    for cc in range(2):
                        nc.tensor.matmul(
                            ps[:, oh * 512 : (oh + 1) * 512],
                            attnT[cc][:, tt * 128 : (tt + 1) * 128],
                            wo_sb[:, cc, oh * 512 : (oh + 1) * 512],
                            start=(cc == 0),
                            stop=(cc == 1),
                        )
                ot = outp.tile([128, D], F32, tag="ot", name="ot")
                nc.vector.tensor_add(ot[:], ps[:], bo_bc[:])
                nc.sync.dma_start(our[tt], ot[:])

            # ---------------- preamble projections ----------------
            # Q chunks for qb0/pair0, all of K (both pairs), first V chunks.
            for tt in range(2):
                qk_chunk(xqr, wq_sb, bq_sb, QT, 0, tt)
            for tt in range(8):
                xt = qk_chunk(xkr, wk_sb, bk_sb, KT, 0, tt)
                qk_chunk(xkr, wk_sb, bk_sb, KT, 1, tt, xt=xt)
            for tcI in range(4):
                v_chunk(tcI)

            # ---------------- filler plan ----------------
            # per-segment (seg = qb*2 + pair) explicit (kt -> item) schedule;
            # items are issued at their kt position inside the attention loop
            # so their emission can never precede that point's QK/exp chain.
            # "fin" = the previous segment's tail finish (rowsum broadcast,
            # reciprocal, normalize) deferred into this segment so its
            # DVE-gated matmuls are emitted after this segment's stream is
            # underway.
            seg_fillers = {s: [] for s in range(2 * NQB)}
            seg_fillers[0] = (
                [(0, ("q", 1, 0)), (2, ("q", 1, 1))]
                + [(max(0, t - 5), ("v", t)) for t in range(4, NKT)]
            )
            seg_fillers[1] = [(1, ("fin",)), (8, ("q", 0, 2)), (12, ("q", 0, 3))]
            seg_fillers[2] = (
                [(1, ("fin",)), (2, ("q", 1, 2)), (4, ("q", 1, 3))]
                + [(6 + 3 * tt, ("o", 0, tt)) for tt in range(8)]
            )
            seg_fillers[3] = [(1, ("fin",)), (8, ("q", 0, 4)), (12, ("q", 0, 5))]
            seg_fillers[4] = (
                [(1, ("fin",)), (2, ("q", 1, 4)), (4, ("q", 1, 5))]
                + [(6 + 3 * tt, ("o", 1, tt)) for tt in range(8)]
            )
            seg_fillers[5] = [(1, ("fin",)), (8, ("q", 0, 6)), (12, ("q", 0, 7))]
            seg_fillers[6] = [(1, ("fin",)), (2, ("q", 1, 6)), (4, ("q", 1, 7))]
            seg_fillers[7] = [(1, ("fin",))] + [
                (6 + 3 * tt, ("o", 2, tt)) for tt in range(8)
            ]

            finish_queue = []

            def run_filler(item):
                kind = item[0]
                if kind == "q":
                    qk_chunk(xqr, wq_sb, bq_sb, QT, item[1], item[2])
                elif kind == "k":
                    qk_chunk(xkr, wk_sb, bk_sb, KT, item[1], item[2])
                elif kind == "v":
                    v_chunk(item[1])
                elif kind == "o":
                    out_chunk(item[1] * 8 + item[2])
                elif kind == "fin":
                    finish_queue.pop(0)()

            # ---------------- attention ----------------
            if masked:
                mrr = maskT_d.rearrange("(kt p) q -> kt p q", p=128)
            for qb in range(NQB):
                q0 = qb * QBLK
                for pair in range(2):
                    seg = qb * 2 + pair
                    fpos = {}
                    for kt_pos, item in seg_fillers[seg]:
                        fpos.setdefault(kt_pos, []).append(item)
                    QTp, KTp, ATp = QT[pair], KT[pair], attnT[pair]
                    attn_ps = avps.tile([128, QBLK], F32, tag="attn", name="attn")
                    rs5 = [None, None]
                    et_cur = [None, None]
                    for kt in range(NKT):
                        for item in fpos.get(kt, ()):
                            run_filler(item)
                        ki = kt % 4
                        sts = [None, None]
                        for h2 in range(2):
                            if ki == 0:
                                et_cur[h2] = etp.tile(
                                    [128, 4, QBLK], BF16, tag="et", name="et"
                                )
                            sts[h2] = stp.tile([128, QBLK], F32, tag="st", name="st")
                        for qh in range(2):
                            for h2 in range(2):
                                b0 = h2 * 64
                                nc.tensor.matmul(
                                    sts[h2][:, qh * 512 : (qh + 1) * 512],
                                    KTp[b0 : b0 + 64, kt * 128 : (kt + 1) * 128],
                                    QTp[b0 : b0 + 64, q0 + qh * 512 : q0 + (qh + 1) * 512],
                                    start=True,
                                    stop=True,
                                )
                        for h2 in range(2):
                            et = et_cur[h2]
                            nc.scalar.activation(
                                et[:, ki, :],
                                sts[h2][:],
                                mybir.ActivationFunctionType.Exp,
                                scale=1.0 / math.sqrt(DH),
                            )
                            if masked:
                                mk = rtmp.tile([128, QBLK], BF16, tag="mk", name="mk")
                                nc.sync.dma_start(mk[:], mrr[kt][:, q0 : q0 + QBLK])
                                nc.vector.tensor_mul(et[:, ki, :], et[:, ki, :], mk[:])
                        # AV accumulate (start on first kt per quadrant)
                        for qh in range(2):
                            for h2 in range(2):
                                b0 = h2 * 64
                                h_local = pair * 2 + h2
                                nc.tensor.matmul(
                                    attn_ps[b0 : b0 + 64, qh * 512 : (qh + 1) * 512],
                                    Vsb[:, kt, h_local * 64 : (h_local + 1) * 64],
                                    et_cur[h2][:, ki, qh * 512 : (qh + 1) * 512],
                                    start=(kt == 0),
                                    stop=(kt == NKT - 1),
                                )
                        # rowsum tree contribution once per 4-k-tile group
                        # (last group deferred past the attn_ps release copy)
                        if ki == 3 and kt != NKT - 1:
                            for h2 in range(2):
                                et = et_cur[h2]
                                t2 = rtmp.tile([128, 2, QBLK], BF16, tag="t2", name="t2")
                                nc.vector.tensor_add(t2[:], et[:, 0:2, :], et[:, 2:4, :])
                                if kt == 3:
                                    rs5[h2] = rtmp.tile(
                                        [128, QBLK], BF16, tag="rs5", name="rs5"
                                    )
                                    nc.vector.tensor_add(rs5[h2][:], t2[:, 0, :], t2[:, 1, :])
                                else:
                                    ts_ = rtmp.tile([128, QBLK], BF16, tag="ts", name="ts")
                                    nc.vector.tensor_add(ts_[:], t2[:, 0, :], t2[:, 1, :])
                                    nc.vector.tensor_add(rs5[h2][:], rs5[h2][:], ts_[:])
                    # ---- pair tail ----
                    # release attn_ps ASAP: cheap DVE copy to SBUF (next
                    # pair's first AV waits on this).
                    atu = atup.tile([128, QBLK], F32, tag="atu", name="atu")
                    nc.vector.tensor_copy(atu[:], attn_ps[:])

                    # everything else (deferred last rowsum-tree group, rowsum
                    # broadcast matmuls, reciprocal, normalize) is deferred
                    # into the NEXT segment as a "fin" filler so its emission
                    # can never stall the next segment's QK/exp ramp.
                    def make_finish(et_pair, rs5_pair, atu_t, ATp_t, q0_t):
                        def finish():
                            for h2 in range(2):
                                et = et_pair[h2]
                                t2 = rtmp.tile([128, 2, QBLK], BF16, tag="t2", name="t2")
                                nc.vector.tensor_add(t2[:], et[:, 0:2, :], et[:, 2:4, :])
                                ts_ = rtmp.tile([128, QBLK], BF16, tag="ts", name="ts")
                                nc.vector.tensor_add(ts_[:], t2[:, 0, :], t2[:, 1, :])
                                nc.vector.tensor_add(
                                    rs5_pair[h2][:], rs5_pair[h2][:], ts_[:]
                                )
                            st_rs = stp.tile([128, QBLK], F32, tag="st", name="st_rs")
                            for qh in range(2):
                                for h2 in range(2):
                                    qx = (qh + h2) % 2
                                    nc.tensor.matmul(
                                        st_rs[h2 * 64 : h2 * 64 + 64,
                                              qx * 512 : qx * 512 + 512],
                                        ones64[:],
                                        rs5_pair[h2][:, qx * 512 : qx * 512 + 512],
                                        start=True,
                                        stop=True,
                                    )
                            rs_bc = rrow.tile([128, QBLK], F32, tag="rsbc", name="rsbc")
                            nc.vector.tensor_copy(rs_bc[:], st_rs[:])
                            for hf in range(2):
                                sl = slice(hf * 512, (hf + 1) * 512)
                                nc.vector.reciprocal(rs_bc[:, sl], rs_bc[:, sl])
                                nc.vector.tensor_tensor(
                                    ATp_t[:, q0_t + hf * 512 : q0_t + (hf + 1) * 512],
                                    atu_t[:, sl],
                                    rs_bc[:, sl],
                                    mybir.AluOpType.mult,
                                )
                        return finish

                    finish_queue.append(make_finish(et_cur, rs5, atu, ATp, q0))
            # tail: finish the last segment, then the last q-block's out-proj
            finish_queue.pop(0)()
            for tt in range(3 * 8, 4 * 8):
                out_chunk(tt)

    return _split_waits(nc) if split_waits else nc


def _prep_in_maps(inputs):
    q = np.asarray(inputs["query"], np.float32)
    k = np.asarray(inputs["key"], np.float32)
    v = np.asarray(inputs["value"], np.float32)
    mask = np.asarray(inputs["mask"])
    Wq = np.asarray(inputs["Wq"], np.float32)
    Wk = np.asarray(inputs["Wk"], np.float32)
    Wv = np.asarray(inputs["Wv"], np.float32)
    Wo = np.asarray(inputs["Wo"], np.float32)
    bq = np.asarray(inputs["bq"], np.float32)
    bk = np.asarray(inputs["bk"], np.float32)
    bv = np.asarray(inputs["bv"], np.float32)
    bo = np.asarray(inputs["bo"], np.float32)

    masked = not bool((mask != 0).all())
    xT = {}
    for nm, x in (("q", q), ("k", k), ("v", v)):
        for b in range(B):
            xT[(nm, b)] = np.ascontiguousarray(x[b].T).astype(bf16)
    if masked:
        maskT = np.ascontiguousarray(
            (np.broadcast_to(mask[0, 0], (S, S)).T != 0)
        ).astype(bf16)

    in_maps = []
    for c in range(NCORES):
        b, hg = c // HG, c % HG
        sl = slice(hg * OC, (hg + 1) * OC)
        m = {
            "xqT": xT[("q", b)],
            "xkT": xT[("k", b)],
            "xvT": xT[("v", b)],
            "wqT": np.ascontiguousarray(Wq[sl].T).astype(bf16),
            "wkT": np.ascontiguousarray(Wk[sl].T).astype(bf16),
            "wvT": np.ascontiguousarray(Wv[sl].T).astype(bf16),
            "bq2": np.ascontiguousarray(bq[sl].reshape(2, 128).T),
            "bk2": np.ascontiguousarray(bk[sl].reshape(2, 128).T),
            "bvr": bv[sl].reshape(1, OC).copy(),
            "woR": np.ascontiguousarray(Wo[:, sl].T).astype(bf16),
            "bor": (bo if hg == 0 else np.zeros_like(bo)).reshape(1, D).copy(),
        }
        if masked:
            m["maskT"] = maskT
        in_maps.append(m)
    return in_maps, masked


def _install_profile_hook():
    """Provide antenv.axon_hooks + register the NTFF profile hook via ctypes
    against libaxon_pjrt.so (the agent image lacks antenv.axon_hooks, which
    makes run_bass_kernel_spmd(trace=True) fall over; see trn_boot.py)."""
    import types
    import ctypes
    import contextlib

    if "antenv.axon_hooks" in sys.modules:
        return
    mod = types.ModuleType("antenv.axon_hooks")
    state = {"hook": None}
    mod.set_axon_ntff_profile_hook = lambda h: state.__setitem__("hook", h)
    mod.get_axon_ntff_profile_hook = lambda: state["hook"]
    sys.modules["antenv.axon_hooks"] = mod

    so_path = "/opt/axon/libaxon_pjrt.so"
    if not os.path.exists(so_path):
        return
    lib = ctypes.CDLL(so_path)
    if not hasattr(lib, "axon_start_nrt_profile"):
        return
    lib.axon_start_nrt_profile.argtypes = [
        ctypes.POINTER(ctypes.c_int64),
        ctypes.c_size_t,
    ]
    lib.axon_start_nrt_profile.restype = ctypes.c_int64
    lib.axon_stop_nrt_profile.argtypes = [ctypes.c_char_p]
    lib.axon_stop_nrt_profile.restype = ctypes.c_int64

    @contextlib.contextmanager
    def _hook(output_dir, device_ids):
        import jax

        jax.devices()
        if device_ids:
            ids = (ctypes.c_int64 * len(device_ids))(*device_ids)
            rc = lib.axon_start_nrt_profile(ids, len(device_ids))
        else:
            rc = lib.axon_start_nrt_profile(None, 0)
        if rc != 0:
            raise RuntimeError(f"axon_start_nrt_profile rc={rc}")
        try:
            yield
        finally:
            n = lib.axon_stop_nrt_profile(str(output_dir).encode())
            print(f"profile: {n} file(s) written to {output_dir}", file=sys.stderr)

    mod.set_axon_ntff_profile_hook(_hook)


def run(inputs, trace=False):
    if trace:
        _install_profile_hook()
    in_maps, masked = _prep_in_maps(inputs)
    nc = _build(masked)
    res = bass_utils.run_bass_kernel_spmd(
        nc, in_maps, core_ids=list(range(NCORES)), trace=trace
    )
    out = np.zeros((B, S, D), np.float32)
    for c in range(NCORES):
        out[c // HG] += res.results[c]["out"]
    return out, res


def kernel(**inputs):
    return run(inputs, trace=False)[0]
